# revision 43
# baseline (speedup 1.0000x reference)
"""Multi-head attention (GAttention) on 8 trn2 NeuronCores.

Reference computation (per batch b):
    q = x @ w_qkv.T            -> [N, 768], heads of 64
    attn = softmax(q k^T / 8)  -> per head [N, M]
    out_h = attn @ v           -> [N, 64]
    out = concat(out_h) @ w_proj.T + b_proj

Sharding: 24 (b, head) units over 8 cores -> each core gets one batch b and
3 heads. Each core computes its heads' attention plus its partial
projection sum [N, 768]; host adds the 4 partials per batch + bias.

The kernel is ScalarE-bound: softmax exp is ACT-only at 1 elem/lane/cycle
(1.2 GHz), 12.6M elems/core ~= 96 us. Everything else is scheduled around
keeping the ACT exp stream dense from ~9 us on:
  1. qproj (bf16) paced by the xT DMA stream, n-half outer so head-0
     queries finish early; wq has the head slice duplicated so qT rows
     64:128 copy rows 0:64 (feeds the row-packed S^T matmuls).
  2. attention in 12 (head, n-quarter) units; per key m-tile PAIR:
       S^T = k q^T   (PE row groups 0/64 concurrently) -> PSUM [128,2,512]
       expT = exp(0.125 S^T) -> SBUF bf16 (ACT, fused scale)
       AV: av[128,512] += va^T expT  (va = [v | ones]; rows 64:128 of av
           hold the softmax denominator), issued LAG iterations behind.
  3. normalize: outTn[0:64] = av[0:64] * recip_approx(av[64:128])
  4. proj (bf16): per 128-row n-tile, 3 heads accumulate in PSUM; output
     DMA'd straight from PSUM to DRAM f32. Proj tiles are interleaved into
     the following attention unit so only the last quarter is tail work.
"""
import numpy as np
import ml_dtypes
from contextlib import ExitStack

import concourse.bass as bass
import concourse.mybir as mybir
import concourse.tile as tile
from concourse import bacc
from concourse.bass_utils import run_bass_kernel_spmd

B, N, DIM = 2, 2048, 768
H, D = 12, 64
M = 2048
NCORES = 8
HPC = 3            # heads per core
NT = N // 128      # 16 query tiles
MT = M // 128      # 16 key tiles
MP = MT // 2       # 8 key-tile pairs
CT = DIM // 128    # 6 contraction tiles for qproj
QN = 512           # attention-unit query granularity (av psum = 1 bank)
NQ = N // QN       # 4 quarters
F32 = mybir.dt.float32
BF16 = mybir.dt.bfloat16
I16 = mybir.dt.int16

# Schraudolph fast-exp constants for the DVE offload path, in bf16
# bit-space: i16 = convert(s * A + B); bitcast(i16) as bf16 ~= exp(0.125*s),
# max rel err ~3%. A = 2^7 * 0.125 * log2(e); B = (127 - 0.0436) * 2^7
# (host-tuned minimax shift).
EXP_A = 23.083120654232846
EXP_B = 16250.4192
# m-tile pairs whose exp runs on the DVE instead of ScalarE
DVE_PAIRS = {2, 5}

_cached = {}
DEBUG_TAPS = False


def build_program():
    nc = bacc.Bacc("TRN2", target_bir_lowering=False, debug=False)
    xT_d = nc.dram_tensor("xT", [128, CT, N], BF16, kind="ExternalInput")
    wq01_d = nc.dram_tensor("wq01", [128, CT, 128], BF16,
                            kind="ExternalInput")
    wq2_d = nc.dram_tensor("wq2", [128, CT, 128], BF16,
                           kind="ExternalInput")
    kT_d = nc.dram_tensor("kT", [128, HPC, MP, 128], BF16,
                          kind="ExternalInput")
    va_d = nc.dram_tensor("va", [128, HPC, MT, 128], BF16,
                          kind="ExternalInput")
    wp01_d = nc.dram_tensor("wp01", [128, DIM], BF16, kind="ExternalInput")
    wp2_d = nc.dram_tensor("wp2", [64, DIM], BF16, kind="ExternalInput")
    out_d = nc.dram_tensor("out", [N, DIM], F32, kind="ExternalOutput")
    if DEBUG_TAPS:
        qTdump_d = nc.dram_tensor("qTdump", [128, HPC, N], BF16,
                                  kind="ExternalOutput")
        otdump_d = nc.dram_tensor("otdump", [128, N], BF16,
                                  kind="ExternalOutput")
        ot2dump_d = nc.dram_tensor("ot2dump", [64, N], BF16,
                                   kind="ExternalOutput")

    with tile.TileContext(nc) as tc, ExitStack() as ctx:
        big = ctx.enter_context(tc.tile_pool(name="big", bufs=1))
        expp = ctx.enter_context(tc.tile_pool(name="expp", bufs=7))
        expi = ctx.enter_context(tc.tile_pool(name="expi", bufs=3))
        nrm = ctx.enter_context(tc.tile_pool(name="nrm", bufs=3))
        stg = ctx.enter_context(tc.tile_pool(name="stg", bufs=3))

        # ACT table warmup: a tiny exp at t~0 so the ~2.7us table load is
        # off the critical path of the first real exp
        wu = big.tile([128, 8], F32)
        nc.gpsimd.memset(wu[:], 0.0)
        wu2 = big.tile([128, 8], F32)
        nc.scalar.activation(wu2[:], wu[:], mybir.ActivationFunctionType.Exp)
        # PE warm-up fodder: dummy bf16 matmuls during the input-DMA wait
        # keep the HAM activity window busy, so qproj and the first
        # attention unit run at 2.4 GHz instead of the cold 1.2 GHz
        wb = big.tile([128, 128], BF16)
        nc.gpsimd.memset(wb[:], 1.0)

        # persistent SBUF tensors; DMA order = consumption order: wq and
        # xT quarter 0 pace qproj(q0), then all k/v (units of quarter 0 run
        # through all 3 heads), then the later xT quarters and wp.
        # va_r (f32 copy for the DVE-exp AV path) is converted on the idle
        # gpsimd engine as each va head lands.
        wq01_t = big.tile([128, CT, 128], BF16)
        nc.sync.dma_start(wq01_t[:], wq01_d[:])
        wq2_t = big.tile([128, CT, 128], BF16)
        nc.sync.dma_start(wq2_t[:], wq2_d[:])
        xT_t = big.tile([128, CT, N], BF16)
        kT_t = big.tile([128, HPC, MP, 128], BF16)
        va_t = big.tile([128, HPC, MT, 128], BF16)

        def _dma_xq(q):
            # one DMA per quarter (6 x 1KB runs per partition) instead of
            # six — descriptor-generation time on the queue is ~600ns apiece
            nc.sync.dma_start(xT_t[:, :, q * QN:(q + 1) * QN],
                              xT_d[:, :, q * QN:(q + 1) * QN])

        def _dma_kv(h):
            nc.sync.dma_start(kT_t[:, h], kT_d[:, h])
            nc.sync.dma_start(va_t[:, h], va_d[:, h])

        _dma_xq(0)
        _dma_kv(0)
        _dma_kv(1)
        _dma_xq(1)
        _dma_kv(2)
        wp01_t = big.tile([128, DIM], BF16)
        nc.sync.dma_start(wp01_t[:], wp01_d[:])
        wp2_t = big.tile([64, DIM], BF16)
        nc.sync.dma_start(wp2_t[:], wp2_d[:])
        _dma_xq(2)
        _dma_xq(3)

        qT_t = big.tile([128, HPC, N], BF16)
        # proj contraction operands: heads 0|1 stacked on the partition dim,
        # head 2 separate
        outTn01_t = big.tile([128, N], BF16)
        outTn2_t = big.tile([64, N], BF16)

        # single instruction stream: per quarter, qproj (into the st psum
        # pool) then the 3 attention units; proj tiles of the previous
        # quarter interleave one per iteration. PSUM: st 2x2 banks +
        # av 2x1 + pj 2x1 + 2 spare.
        with tc.tile_pool(name="st_ps", bufs=2, space="PSUM") as st_ps, \
             tc.tile_pool(name="av_ps", bufs=2, space="PSUM") as av_ps, \
             tc.tile_pool(name="pj_ps", bufs=2, space="PSUM") as pj_ps:
            av_by_unit = {}
            pend = []
            proj_todo = []
            LAG = 2

            def _av(pd):
                (h, q), et0, et1, p, first, last = pd
                av = av_by_unit[(h, q)]
                nc.tensor.matmul(av[:], va_t[:, h, 2 * p], et0,
                                 start=first, stop=False)
                nc.tensor.matmul(av[:], va_t[:, h, 2 * p + 1], et1,
                                 start=False, stop=last)

            def _norm(unit):
                # denominator copied to a base-partition-0 SBUF tile first:
                # the custom-DVE recip misbehaves on HW when its input AP
                # sits at a partition offset (sim-only correct)
                h, q = unit
                av = av_by_unit.pop(unit)
                nsl = slice(q * QN, (q + 1) * QN)
                dn = nrm.tile([64, QN], F32, tag="dn", name="dn")
                nc.vector.tensor_copy(dn[:], av[64:128, :])
                rs = nrm.tile([64, QN], F32, tag="rs", name="rs")
                nc.vector.reciprocal_approx_fast(rs[:], dn[:])
                if h == 0:
                    dst = outTn01_t[0:64, nsl]
                elif h == 1:
                    dst = outTn01_t[64:128, nsl]
                else:
                    dst = outTn2_t[:, nsl]
                nc.vector.tensor_mul(dst, av[0:64, :], rs[:])

            qp_by = {}

            def _qproj_part(q, grp, part):
                # q projection for one (quarter, head-group), 2 c-tiles per
                # call so the PE bubble it injects into the attention stream
                # stays small; accumulates in a pj-pool buffer. grp 0 stacks
                # heads 0|1 in the stationary free dim (no duplication); the
                # copies fan the halves out into qT's duplicated layout.
                if part == 0:
                    qp_by[(q, grp)] = pj_ps.tile([128, 512], F32,
                                                 tag="pp", name="qp")
                qp = qp_by[(q, grp)]
                wq_t = wq01_t if grp == 0 else wq2_t
                for c in (2 * part, 2 * part + 1):
                    nc.tensor.matmul(
                        qp[:], wq_t[:, c],
                        xT_t[:, c, q * QN:(q + 1) * QN],
                        start=(c == 0), stop=(c == CT - 1),
                    )
                if part == 2:
                    nsl = slice(q * QN, (q + 1) * QN)
                    if grp == 0:
                        nc.vector.tensor_copy(qT_t[0:64, 0, nsl], qp[0:64])
                        nc.vector.tensor_copy(qT_t[64:128, 0, nsl], qp[0:64])
                        nc.vector.tensor_copy(qT_t[0:64, 1, nsl], qp[64:128])
                        nc.vector.tensor_copy(qT_t[64:128, 1, nsl],
                                              qp[64:128])
                    else:
                        nc.vector.tensor_copy(qT_t[:, 2, nsl], qp[:])
                    del qp_by[(q, grp)]

            def _proj_half(q, j, oc):
                # one 128-row n-tile x 384 out-cols; heads 0|1 via a single
                # 128-deep contraction, head 2 accumulated on top
                nn = (q * 4 + j) * 128
                osl = slice(oc * 384, (oc + 1) * 384)
                pp = pj_ps.tile([128, 512], F32, tag="pp", name="pp")
                nc.tensor.matmul(pp[:, 0:384], outTn01_t[:, nn:nn + 128],
                                 wp01_t[:, osl], start=True, stop=False)
                nc.tensor.matmul(pp[:, 0:384], outTn2_t[:, nn:nn + 128],
                                 wp2_t[:, osl], start=False, stop=True,
                                 tile_position=(0, 0))
                os_t = stg.tile([128, 384], F32, tag="os", name="os")
                if oc == 0:
                    nc.vector.tensor_copy(os_t[:], pp[:, 0:384])
                else:
                    nc.scalar.copy(os_t[:], pp[:, 0:384])
                nc.sync.dma_start(out_d[nn:nn + 128, osl], os_t[:])

            def _flush(limit):
                while len(pend) > limit:
                    pd = pend.pop(0)
                    _av(pd)
                    if pd[5]:
                        _norm(pd[0])
                        h, q = pd[0]
                        if h == HPC - 1:
                            proj_todo.extend(
                                (q, j, oc) for j in range(4) for oc in range(2))

            wst = st_ps.tile([128, 2, 512], F32, tag="st", name="warm")
            for i in range(56):
                nc.tensor.matmul(wst[:, 0, 0:128], wb[:], wb[:],
                                 start=True, stop=True)
            for grp in range(2):
                for part in range(3):
                    _qproj_part(0, grp, part)
            iters = [(h, q, p)
                     for q in range(NQ) for h in range(HPC) for p in range(MP)]
            for idx, (h, q, p) in enumerate(iters):
                unit = (h, q)
                if unit not in av_by_unit:
                    av_by_unit[unit] = av_ps.tile(
                        [128, QN], F32, tag="av", name="av")
                n0 = q * QN
                st = st_ps.tile([128, 2, 512], F32, tag="st", name="st")
                nc.tensor.matmul(
                    st[:, 0], kT_t[0:64, h, p], qT_t[0:64, h, n0:n0 + QN],
                    start=True, stop=True, tile_position=(0, 0),
                )
                nc.tensor.matmul(
                    st[:, 1], kT_t[64:128, h, p], qT_t[64:128, h, n0:n0 + QN],
                    start=True, stop=True, tile_position=(64, 0),
                )
                # prefill: don't issue AVs behind the first unit's S^T/exp
                # stream, so it isn't queue-blocked on the va DMA; drain the
                # backlog one entry per iteration afterwards
                _flush(max(LAG - 1, 15 - idx))
                if p in DVE_PAIRS:
                    eti = expi.tile([128, 2, 512], I16, tag="eti", name="eti")
                    nc.vector.tensor_scalar(
                        eti[:], st[:], EXP_A, EXP_B,
                        mybir.AluOpType.mult, mybir.AluOpType.add)
                    et0 = eti[:, 0].bitcast(BF16)
                    et1 = eti[:, 1].bitcast(BF16)
                else:
                    et = expp.tile([128, 2, 512], BF16, tag="et", name="et")
                    nc.scalar.activation(
                        et[:], st[:], mybir.ActivationFunctionType.Exp,
                        scale=float(D) ** -0.5,
                    )
                    et0, et1 = et[:, 0], et[:, 1]
                pend.append((unit, et0, et1, p, p == 0, p == MP - 1))
                if q + 1 < NQ and p >= MP - 3 and h < 2:
                    _qproj_part(q + 1, h, p - (MP - 3))
                if proj_todo:
                    _proj_half(*proj_todo.pop(0))
            _flush(0)
            while proj_todo:
                _proj_half(*proj_todo.pop(0))
            if DEBUG_TAPS:
                nc.sync.dma_start(qTdump_d[:], qT_t[:])
                nc.sync.dma_start(otdump_d[:], outTn01_t[:])
                nc.sync.dma_start(ot2dump_d[:], outTn2_t[:])

    nc.compile()
    return nc


def build_in_maps(x, k, v, w_qkv, w_proj):
    x = np.asarray(x, dtype=np.float32)
    k = np.asarray(k, dtype=np.float32)
    v = np.asarray(v, dtype=np.float32)
    wqT = np.ascontiguousarray(np.asarray(w_qkv, np.float32).T)   # [C, 768]
    wpT = np.ascontiguousarray(np.asarray(w_proj, np.float32).T)  # [768, 768]
    bf = ml_dtypes.bfloat16

    in_maps = []
    for core in range(NCORES):
        b = core // 4
        hs = [3 * (core % 4) + i for i in range(HPC)]
        # xT [128, CT, N]: partition-major layout of x[b].T
        xT = np.ascontiguousarray(
            x[b].T.reshape(CT, 128, N).transpose(1, 0, 2).astype(bf))
        # wq01 [128, CT, 128]: heads 0|1 stacked in the output columns;
        # wq2: head 2 duplicated -> qT rows 64:128 == rows 0:64
        b0 = wqT[:, 64 * hs[0]:64 * hs[0] + 64]
        b1 = wqT[:, 64 * hs[1]:64 * hs[1] + 64]
        b2 = wqT[:, 64 * hs[2]:64 * hs[2] + 64]
        wq01 = (np.concatenate([b0, b1], axis=1)
                .reshape(CT, 128, 128).transpose(1, 0, 2).astype(bf))
        wq2 = (np.concatenate([b2, b2], axis=1)
               .reshape(CT, 128, 128).transpose(1, 0, 2).astype(bf))
        # kT [128, HPC, MP, 128]: rows 0:64 = head-dim of even m-tile,
        # rows 64:128 = head-dim of odd m-tile of each pair
        kb = k[b, hs].astype(bf)                            # [3, M, D]
        kT = np.empty((128, HPC, MP, 128), dtype=bf)
        for hi in range(HPC):
            for p in range(MP):
                kT[0:64, hi, p, :] = kb[hi, 256 * p:256 * p + 128, :].T
                kT[64:128, hi, p, :] = kb[hi, 256 * p + 128:256 * p + 256, :].T
        # va [128, HPC, MT, 128]: [v | ones]; partition = key-within-tile
        va = np.ones((128, HPC, MT, 128), dtype=bf)
        va[:, :, :, :D] = (
            v[b, hs].reshape(HPC, MT, 128, D).transpose(2, 0, 1, 3).astype(bf))
        # wp01 [128, DIM]: heads 0|1 stacked on partitions; wp2 [64, DIM]
        wp01 = np.empty((128, DIM), dtype=bf)
        wp01[0:64] = wpT[64 * hs[0]:64 * hs[0] + 64, :].astype(bf)
        wp01[64:128] = wpT[64 * hs[1]:64 * hs[1] + 64, :].astype(bf)
        wp2 = np.ascontiguousarray(
            wpT[64 * hs[2]:64 * hs[2] + 64, :].astype(bf))
        in_maps.append({"xT": xT,
                        "wq01": np.ascontiguousarray(wq01),
                        "wq2": np.ascontiguousarray(wq2),
                        "kT": np.ascontiguousarray(kT),
                        "va": np.ascontiguousarray(va),
                        "wp01": wp01, "wp2": wp2})
    return in_maps


def kernel(x, k, v, w_qkv, w_proj, b_proj):
    b_proj = np.asarray(b_proj, dtype=np.float32)

    if "nc" not in _cached:
        _cached["nc"] = build_program()
    nc = _cached["nc"]

    in_maps = build_in_maps(x, k, v, w_qkv, w_proj)
    res = run_bass_kernel_spmd(nc, in_maps, core_ids=list(range(NCORES)))

    out = np.empty((B, N, DIM), dtype=np.float32)
    for b in range(B):
        acc = np.zeros((N, DIM), dtype=np.float64)
        for core in range(4 * b, 4 * b + 4):
            acc += res.results[core]["out"]
        out[b] = (acc + b_proj).astype(np.float32)
    return out


# revision 44
# speedup vs baseline: 1.0061x; 1.0061x over previous
"""Multi-head attention (GAttention) on 8 trn2 NeuronCores.

Reference computation (per batch b):
    q = x @ w_qkv.T            -> [N, 768], heads of 64
    attn = softmax(q k^T / 8)  -> per head [N, M]
    out_h = attn @ v           -> [N, 64]
    out = concat(out_h) @ w_proj.T + b_proj

Sharding: 24 (b, head) units over 8 cores -> each core gets one batch b and
3 heads. Each core computes its heads' attention plus its partial
projection sum [N, 768]; host adds the 4 partials per batch + bias.

The kernel is ScalarE-bound: softmax exp is ACT-only at 1 elem/lane/cycle
(1.2 GHz), 12.6M elems/core ~= 96 us. Everything else is scheduled around
keeping the ACT exp stream dense from ~9 us on:
  1. qproj (bf16) paced by the xT DMA stream, n-half outer so head-0
     queries finish early; wq has the head slice duplicated so qT rows
     64:128 copy rows 0:64 (feeds the row-packed S^T matmuls).
  2. attention in 12 (head, n-quarter) units; per key m-tile PAIR:
       S^T = k q^T   (PE row groups 0/64 concurrently) -> PSUM [128,2,512]
       expT = exp(0.125 S^T) -> SBUF bf16 (ACT, fused scale)
       AV: av[128,512] += va^T expT  (va = [v | ones]; rows 64:128 of av
           hold the softmax denominator), issued LAG iterations behind.
  3. normalize: outTn[0:64] = av[0:64] * recip_approx(av[64:128])
  4. proj (bf16): per 128-row n-tile, 3 heads accumulate in PSUM; output
     DMA'd straight from PSUM to DRAM f32. Proj tiles are interleaved into
     the following attention unit so only the last quarter is tail work.
"""
import numpy as np
import ml_dtypes
from contextlib import ExitStack

import concourse.bass as bass
import concourse.mybir as mybir
import concourse.tile as tile
from concourse import bacc
from concourse.bass_utils import run_bass_kernel_spmd

B, N, DIM = 2, 2048, 768
H, D = 12, 64
M = 2048
NCORES = 8
HPC = 3            # heads per core
NT = N // 128      # 16 query tiles
MT = M // 128      # 16 key tiles
MP = MT // 2       # 8 key-tile pairs
CT = DIM // 128    # 6 contraction tiles for qproj
QN = 512           # attention-unit query granularity (av psum = 1 bank)
NQ = N // QN       # 4 quarters
F32 = mybir.dt.float32
BF16 = mybir.dt.bfloat16
I16 = mybir.dt.int16

# Schraudolph fast-exp constants for the DVE offload path, in bf16
# bit-space: i16 = convert(s * A + B); bitcast(i16) as bf16 ~= exp(0.125*s),
# max rel err ~3%. A = 2^7 * 0.125 * log2(e); B = (127 - 0.0436) * 2^7
# (host-tuned minimax shift).
EXP_A = 23.083120654232846
EXP_B = 16250.4192
# m-tile pairs whose exp runs on the DVE instead of ScalarE
DVE_PAIRS = {2, 5}

_cached = {}
DEBUG_TAPS = False


def build_program():
    nc = bacc.Bacc("TRN2", target_bir_lowering=False, debug=False)
    xT_d = nc.dram_tensor("xT", [DIM, N], BF16, kind="ExternalInput")
    wq01_d = nc.dram_tensor("wq01", [128, CT, 128], BF16,
                            kind="ExternalInput")
    wq2_d = nc.dram_tensor("wq2", [128, CT, 128], BF16,
                           kind="ExternalInput")
    kT_d = nc.dram_tensor("kT", [128, HPC, MP, 128], BF16,
                          kind="ExternalInput")
    va_d = nc.dram_tensor("va", [128, HPC, MT, 128], BF16,
                          kind="ExternalInput")
    wp01_d = nc.dram_tensor("wp01", [128, DIM], BF16, kind="ExternalInput")
    wp2_d = nc.dram_tensor("wp2", [64, DIM], BF16, kind="ExternalInput")
    out_d = nc.dram_tensor("out", [N, DIM], F32, kind="ExternalOutput")
    if DEBUG_TAPS:
        qTdump_d = nc.dram_tensor("qTdump", [128, HPC, N], BF16,
                                  kind="ExternalOutput")
        otdump_d = nc.dram_tensor("otdump", [128, N], BF16,
                                  kind="ExternalOutput")
        ot2dump_d = nc.dram_tensor("ot2dump", [64, N], BF16,
                                   kind="ExternalOutput")

    with tile.TileContext(nc) as tc, ExitStack() as ctx:
        big = ctx.enter_context(tc.tile_pool(name="big", bufs=1))
        expp = ctx.enter_context(tc.tile_pool(name="expp", bufs=7))
        expi = ctx.enter_context(tc.tile_pool(name="expi", bufs=3))
        nrm = ctx.enter_context(tc.tile_pool(name="nrm", bufs=3))
        stg = ctx.enter_context(tc.tile_pool(name="stg", bufs=3))

        # ACT table warmup: a tiny exp at t~0 so the ~2.7us table load is
        # off the critical path of the first real exp
        wu = big.tile([128, 8], F32)
        nc.gpsimd.memset(wu[:], 0.0)
        wu2 = big.tile([128, 8], F32)
        nc.scalar.activation(wu2[:], wu[:], mybir.ActivationFunctionType.Exp)

        # persistent SBUF tensors; DMA order = consumption order: wq and
        # xT quarter 0 pace qproj(q0), then all k/v (units of quarter 0 run
        # through all 3 heads), then the later xT quarters and wp.
        # va_r (f32 copy for the DVE-exp AV path) is converted on the idle
        # gpsimd engine as each va head lands.
        wq01_t = big.tile([128, CT, 128], BF16)
        nc.sync.dma_start(wq01_t[:], wq01_d[:])
        wq2_t = big.tile([128, CT, 128], BF16)
        nc.sync.dma_start(wq2_t[:], wq2_d[:])
        xT_t = [big.tile([128, N], BF16, name=f"xT{c}", tag=f"xT{c}")
                for c in range(CT)]
        kT_t = big.tile([128, HPC, MP, 128], BF16)
        va_t = big.tile([128, HPC, MT, 128], BF16)

        def _dma_xq(q):
            for c in range(CT):
                nc.sync.dma_start(xT_t[c][:, q * QN:(q + 1) * QN],
                                  xT_d[c * 128:(c + 1) * 128,
                                       q * QN:(q + 1) * QN])

        def _dma_kv(h):
            nc.sync.dma_start(kT_t[:, h], kT_d[:, h])
            nc.sync.dma_start(va_t[:, h], va_d[:, h])

        _dma_xq(0)
        _dma_kv(0)
        _dma_kv(1)
        _dma_xq(1)
        _dma_kv(2)
        wp01_t = big.tile([128, DIM], BF16)
        nc.sync.dma_start(wp01_t[:], wp01_d[:])
        wp2_t = big.tile([64, DIM], BF16)
        nc.sync.dma_start(wp2_t[:], wp2_d[:])
        _dma_xq(2)
        _dma_xq(3)

        qT_t = big.tile([128, HPC, N], BF16)
        # proj contraction operands: heads 0|1 stacked on the partition dim,
        # head 2 separate
        outTn01_t = big.tile([128, N], BF16)
        outTn2_t = big.tile([64, N], BF16)

        # single instruction stream: per quarter, qproj (into the st psum
        # pool) then the 3 attention units; proj tiles of the previous
        # quarter interleave one per iteration. PSUM: st 2x2 banks +
        # av 2x1 + pj 2x1 + 2 spare.
        with tc.tile_pool(name="st_ps", bufs=2, space="PSUM") as st_ps, \
             tc.tile_pool(name="av_ps", bufs=2, space="PSUM") as av_ps, \
             tc.tile_pool(name="pj_ps", bufs=2, space="PSUM") as pj_ps:
            av_by_unit = {}
            pend = []
            proj_todo = []
            LAG = 2

            def _av(pd):
                (h, q), et0, et1, p, first, last = pd
                av = av_by_unit[(h, q)]
                nc.tensor.matmul(av[:], va_t[:, h, 2 * p], et0,
                                 start=first, stop=False)
                nc.tensor.matmul(av[:], va_t[:, h, 2 * p + 1], et1,
                                 start=False, stop=last)

            def _norm(unit):
                # denominator copied to a base-partition-0 SBUF tile first:
                # the custom-DVE recip misbehaves on HW when its input AP
                # sits at a partition offset (sim-only correct)
                h, q = unit
                av = av_by_unit.pop(unit)
                nsl = slice(q * QN, (q + 1) * QN)
                dn = nrm.tile([64, QN], F32, tag="dn", name="dn")
                nc.vector.tensor_copy(dn[:], av[64:128, :])
                rs = nrm.tile([64, QN], F32, tag="rs", name="rs")
                nc.vector.reciprocal_approx_fast(rs[:], dn[:])
                if h == 0:
                    dst = outTn01_t[0:64, nsl]
                elif h == 1:
                    dst = outTn01_t[64:128, nsl]
                else:
                    dst = outTn2_t[:, nsl]
                nc.vector.tensor_mul(dst, av[0:64, :], rs[:])

            qp_by = {}

            def _qproj_part(q, grp, part):
                # q projection for one (quarter, head-group), 2 c-tiles per
                # call so the PE bubble it injects into the attention stream
                # stays small; accumulates in a pj-pool buffer. grp 0 stacks
                # heads 0|1 in the stationary free dim (no duplication); the
                # copies fan the halves out into qT's duplicated layout.
                if part == 0:
                    qp_by[(q, grp)] = pj_ps.tile([128, 512], F32,
                                                 tag="pp", name="qp")
                qp = qp_by[(q, grp)]
                wq_t = wq01_t if grp == 0 else wq2_t
                for c in (2 * part, 2 * part + 1):
                    nc.tensor.matmul(
                        qp[:], wq_t[:, c],
                        xT_t[c][:, q * QN:(q + 1) * QN],
                        start=(c == 0), stop=(c == CT - 1),
                    )
                if part == 2:
                    nsl = slice(q * QN, (q + 1) * QN)
                    if grp == 0:
                        nc.vector.tensor_copy(qT_t[0:64, 0, nsl], qp[0:64])
                        nc.vector.tensor_copy(qT_t[64:128, 0, nsl], qp[0:64])
                        nc.vector.tensor_copy(qT_t[0:64, 1, nsl], qp[64:128])
                        nc.vector.tensor_copy(qT_t[64:128, 1, nsl],
                                              qp[64:128])
                    else:
                        nc.vector.tensor_copy(qT_t[:, 2, nsl], qp[:])
                    del qp_by[(q, grp)]

            def _proj_half(q, j, oc):
                # one 128-row n-tile x 384 out-cols; heads 0|1 via a single
                # 128-deep contraction, head 2 accumulated on top
                nn = (q * 4 + j) * 128
                osl = slice(oc * 384, (oc + 1) * 384)
                pp = pj_ps.tile([128, 512], F32, tag="pp", name="pp")
                nc.tensor.matmul(pp[:, 0:384], outTn01_t[:, nn:nn + 128],
                                 wp01_t[:, osl], start=True, stop=False)
                nc.tensor.matmul(pp[:, 0:384], outTn2_t[:, nn:nn + 128],
                                 wp2_t[:, osl], start=False, stop=True,
                                 tile_position=(0, 0))
                os_t = stg.tile([128, 384], F32, tag="os", name="os")
                if oc == 0:
                    nc.vector.tensor_copy(os_t[:], pp[:, 0:384])
                else:
                    nc.scalar.copy(os_t[:], pp[:, 0:384])
                nc.sync.dma_start(out_d[nn:nn + 128, osl], os_t[:])

            def _flush(limit):
                while len(pend) > limit:
                    pd = pend.pop(0)
                    _av(pd)
                    if pd[5]:
                        _norm(pd[0])
                        h, q = pd[0]
                        if h == HPC - 1:
                            proj_todo.extend(
                                (q, j, oc) for j in range(4) for oc in range(2))

            for grp in range(2):
                for part in range(3):
                    _qproj_part(0, grp, part)
            iters = [(h, q, p)
                     for q in range(NQ) for h in range(HPC) for p in range(MP)]
            for idx, (h, q, p) in enumerate(iters):
                unit = (h, q)
                if unit not in av_by_unit:
                    av_by_unit[unit] = av_ps.tile(
                        [128, QN], F32, tag="av", name="av")
                n0 = q * QN
                st = st_ps.tile([128, 2, 512], F32, tag="st", name="st")
                nc.tensor.matmul(
                    st[:, 0], kT_t[0:64, h, p], qT_t[0:64, h, n0:n0 + QN],
                    start=True, stop=True, tile_position=(0, 0),
                )
                nc.tensor.matmul(
                    st[:, 1], kT_t[64:128, h, p], qT_t[64:128, h, n0:n0 + QN],
                    start=True, stop=True, tile_position=(64, 0),
                )
                # prefill: don't issue AVs behind the first unit's S^T/exp
                # stream, so it isn't queue-blocked on the va DMA; drain the
                # backlog one entry per iteration afterwards
                _flush(max(LAG - 1, 15 - idx))
                if p in DVE_PAIRS:
                    eti = expi.tile([128, 2, 512], I16, tag="eti", name="eti")
                    nc.vector.tensor_scalar(
                        eti[:], st[:], EXP_A, EXP_B,
                        mybir.AluOpType.mult, mybir.AluOpType.add)
                    et0 = eti[:, 0].bitcast(BF16)
                    et1 = eti[:, 1].bitcast(BF16)
                else:
                    et = expp.tile([128, 2, 512], BF16, tag="et", name="et")
                    nc.scalar.activation(
                        et[:], st[:], mybir.ActivationFunctionType.Exp,
                        scale=float(D) ** -0.5,
                    )
                    et0, et1 = et[:, 0], et[:, 1]
                pend.append((unit, et0, et1, p, p == 0, p == MP - 1))
                if q + 1 < NQ and p >= MP - 3 and h < 2:
                    _qproj_part(q + 1, h, p - (MP - 3))
                if proj_todo:
                    _proj_half(*proj_todo.pop(0))
            _flush(0)
            while proj_todo:
                _proj_half(*proj_todo.pop(0))
            if DEBUG_TAPS:
                nc.sync.dma_start(qTdump_d[:], qT_t[:])
                nc.sync.dma_start(otdump_d[:], outTn01_t[:])
                nc.sync.dma_start(ot2dump_d[:], outTn2_t[:])

    nc.compile()
    return nc


def build_in_maps(x, k, v, w_qkv, w_proj):
    x = np.asarray(x, dtype=np.float32)
    k = np.asarray(k, dtype=np.float32)
    v = np.asarray(v, dtype=np.float32)
    wqT = np.ascontiguousarray(np.asarray(w_qkv, np.float32).T)   # [C, 768]
    wpT = np.ascontiguousarray(np.asarray(w_proj, np.float32).T)  # [768, 768]
    bf = ml_dtypes.bfloat16

    in_maps = []
    for core in range(NCORES):
        b = core // 4
        hs = [3 * (core % 4) + i for i in range(HPC)]
        xT = np.ascontiguousarray(x[b].T.astype(bf))
        # wq01 [128, CT, 128]: heads 0|1 stacked in the output columns;
        # wq2: head 2 duplicated -> qT rows 64:128 == rows 0:64
        b0 = wqT[:, 64 * hs[0]:64 * hs[0] + 64]
        b1 = wqT[:, 64 * hs[1]:64 * hs[1] + 64]
        b2 = wqT[:, 64 * hs[2]:64 * hs[2] + 64]
        wq01 = (np.concatenate([b0, b1], axis=1)
                .reshape(CT, 128, 128).transpose(1, 0, 2).astype(bf))
        wq2 = (np.concatenate([b2, b2], axis=1)
               .reshape(CT, 128, 128).transpose(1, 0, 2).astype(bf))
        # kT [128, HPC, MP, 128]: rows 0:64 = head-dim of even m-tile,
        # rows 64:128 = head-dim of odd m-tile of each pair
        kb = k[b, hs].astype(bf)                            # [3, M, D]
        kT = np.empty((128, HPC, MP, 128), dtype=bf)
        for hi in range(HPC):
            for p in range(MP):
                kT[0:64, hi, p, :] = kb[hi, 256 * p:256 * p + 128, :].T
                kT[64:128, hi, p, :] = kb[hi, 256 * p + 128:256 * p + 256, :].T
        # va [128, HPC, MT, 128]: [v | ones]; partition = key-within-tile
        va = np.ones((128, HPC, MT, 128), dtype=bf)
        va[:, :, :, :D] = (
            v[b, hs].reshape(HPC, MT, 128, D).transpose(2, 0, 1, 3).astype(bf))
        # wp01 [128, DIM]: heads 0|1 stacked on partitions; wp2 [64, DIM]
        wp01 = np.empty((128, DIM), dtype=bf)
        wp01[0:64] = wpT[64 * hs[0]:64 * hs[0] + 64, :].astype(bf)
        wp01[64:128] = wpT[64 * hs[1]:64 * hs[1] + 64, :].astype(bf)
        wp2 = np.ascontiguousarray(
            wpT[64 * hs[2]:64 * hs[2] + 64, :].astype(bf))
        in_maps.append({"xT": xT,
                        "wq01": np.ascontiguousarray(wq01),
                        "wq2": np.ascontiguousarray(wq2),
                        "kT": np.ascontiguousarray(kT),
                        "va": np.ascontiguousarray(va),
                        "wp01": wp01, "wp2": wp2})
    return in_maps


def kernel(x, k, v, w_qkv, w_proj, b_proj):
    b_proj = np.asarray(b_proj, dtype=np.float32)

    if "nc" not in _cached:
        _cached["nc"] = build_program()
    nc = _cached["nc"]

    in_maps = build_in_maps(x, k, v, w_qkv, w_proj)
    res = run_bass_kernel_spmd(nc, in_maps, core_ids=list(range(NCORES)))

    out = np.empty((B, N, DIM), dtype=np.float32)
    for b in range(B):
        acc = np.zeros((N, DIM), dtype=np.float64)
        for core in range(4 * b, 4 * b + 4):
            acc += res.results[core]["out"]
        out[b] = (acc + b_proj).astype(np.float32)
    return out
